# revision 30
# baseline (speedup 1.0000x reference)
"""MultiHeadAttention (B=2, S=2048, D=1024, H=16) on 8 TRN2 NeuronCores.

Sharding: core c -> batch b = c//4, head-group g = c%4 (4 heads = 256 channels).
Each core computes its 4 heads' attention for its batch plus the partial
out-projection (out_w columns for its channel group); host sums the 4 partials
per batch and adds out_b.

v3 (from the 359us v2):
  - both heads' logits for a key-chunk pair land in ONE [128,2048] 4-bank
    PSUM tile -> a single N=2048 ACTIVATE per kc-pair (25% less Scalar time
    than 2xN=1024, half the ACT semaphore overhead), and the four logits
    matmuls share identical deps so the scheduler keeps the row-packed
    (tile_position 0/64) pairs adjacent -> they run concurrently on the PE
    (v2's per-head exp chains made the scheduler serialize the pairs).
  - phase A is folded into phase B: K(jc0)+Q(jc0,st0) projections run first
    (DMA ring reordered weights-first), attention block 0 starts ~30us in,
    and the remaining K(jc1)/Q chains + all V chunks trickle through the
    PSUM 'op'/'vps' tags inside the attention blocks' PE slack, instead of
    an 82us serial cold-clock phase A.
  - Q/K path fp32r (logits precision), V/exp-out/AV/attn-out/out-proj bf16;
    softmax denominators via the Vaug ones-column + K=2 block-ones broadcast
    matmul + reciprocal_approx_fast.
"""

import os
import sys

import numpy as np

for _p in ("/opt/trn_rl_repo",):
    if os.path.isdir(_p) and _p not in sys.path:
        sys.path.insert(0, _p)

from contextlib import ExitStack

import ml_dtypes

import concourse.bass as bass
import concourse.tile as tile
from concourse import bacc, mybir
from concourse._compat import with_exitstack
from concourse.bass_utils import run_bass_kernel_spmd

B, S, D = 2, 2048, 1024
H = 16
HD = 64
NCORES = 8
JG = 256          # channels per core (4 heads)
DC = D // 128     # 8 contraction chunks
QT_TILES = 4      # 4 x 512 query tiles
KC = S // 128     # 16 key chunks
VW = 65           # V columns per head incl. ones column
FP32 = mybir.dt.float32
FP32R = mybir.dt.float32r
BF16 = mybir.dt.bfloat16
EXP = mybir.ActivationFunctionType.Exp


@with_exitstack
def mha_core_kernel(ctx: ExitStack, tc: tile.TileContext,
                    out, xT, wqT, wkT, wvT, bq, bk, bv, owT):
    nc = tc.nc
    ctx.enter_context(nc.allow_low_precision("bf16 V/AV/out-proj path"))

    p = ctx.enter_context(tc.tile_pool(name="p", bufs=1))
    ps = ctx.enter_context(tc.tile_pool(name="ps", bufs=1, space="PSUM"))

    # QT/KT in bf16: the logits matmuls' SBUF streams are the largest PE
    # power draw (HAM duty), and bf16 Q/K costs ~9e-3 rel err (numpy-
    # verified 'fp32_x_qkvw' config) vs the 2e-2 gate.
    QT_sb = p.tile((128, 2 * S), BF16)
    KT_sb = p.tile((128, 2 * S), BF16)
    Vaug_sb = p.tile((128, KC * 4 * VW), BF16)
    attn_outT_sb = p.tile((128, 2 * S), BF16)
    owT_sb = p.tile((128, 2 * D), BF16)
    ones2_sb = p.tile((2, 128), FP32R)
    xT_sb = p.tile((128, DC * S), FP32R)
    wqT_sb = p.tile((128, DC * JG), FP32R)
    wkT_sb = p.tile((128, DC * JG), FP32R)
    wvT_sb = p.tile((128, DC * JG), FP32R)
    bq_sb = p.tile((128, 2), FP32)
    bk_sb = p.tile((128, 2), FP32)
    bv_bc = p.tile((128, JG), FP32)
    bv_row = p.tile((1, JG), FP32R)
    ones1_sb = p.tile((1, 128), FP32R)
    ones_f32 = p.tile((128, 64), FP32)
    ones2_f32 = p.tile((2, 128), FP32)

    # Stage fp32 ones and DVE-copy (cast) into the bf16 ones columns of
    # Vaug + the fp32r 2-row block-ones used for the denominator bcast.
    nc.vector.memset(ones_f32, 1.0)
    nc.vector.tensor_copy(Vaug_sb[:, HD::VW], ones_f32)
    nc.vector.memset(ones2_f32, 0.0)
    nc.vector.memset(ones2_f32[0:1, 0:64], 1.0)
    # DVE memset can't start at partition 1; DMA-copy the ones block.
    nc.sync.dma_start(out=ones2_f32[1:2, 64:128], in_=ones2_f32[0:1, 0:64])
    nc.vector.tensor_copy(ones2_sb, ones2_f32)
    nc.vector.tensor_copy(ones1_sb[:, 0:64], ones_f32[0:1, 0:64])
    nc.vector.tensor_copy(ones1_sb[:, 64:128], ones_f32[0:1, 0:64])

    # DMA: everything on the sync HWDGE ring (the gpsimd software ring
    # measured only ~50GB/s -- putting wk there gated the whole kernel).
    # The weights are single 3D transfers ([p][dc][col] so each partition
    # line is a contiguous 8KB instead of 8x1KB strided chunks). Order:
    # wk+wq before x (the K/Q chains chase the x chunks), wv mid-x (first
    # V chunk isn't needed until the first attention block), owT last.
    def w3d(w):
        return bass.AP(tensor=w.tensor, offset=w.offset,
                       ap=[[JG, 128], [128 * JG, DC], [1, JG]])

    # x split across BOTH HWDGE rings (sync + scalar run in parallel up to
    # ~425GB/s combined): even dc chunks + wk + wv on sync, odd chunks +
    # wq + smalls + owT on the scalar ring (idle until the first exp).
    nc.sync.dma_start(out=wkT_sb[:, 0:DC * JG], in_=w3d(wkT).bitcast(FP32R))
    nc.scalar.dma_start(out=wqT_sb[:, 0:DC * JG], in_=w3d(wqT).bitcast(FP32R))
    for jc in range(2):
        nc.scalar.dma_start(out=bk_sb[:, jc:jc + 1],
                            in_=bk[jc * 128:(jc + 1) * 128])
        nc.scalar.dma_start(out=bq_sb[:, jc:jc + 1],
                            in_=bq[jc * 128:(jc + 1) * 128])
    nc.scalar.dma_start(out=bv_row, in_=bv.bitcast(FP32R))
    for dc in range(DC):
        ring = nc.sync if dc % 2 == 0 else nc.scalar
        ring.dma_start(out=xT_sb[:, dc * S:(dc + 1) * S],
                       in_=xT[dc * 128:(dc + 1) * 128, :].bitcast(FP32R))
        if dc == 3:
            nc.sync.dma_start(out=wvT_sb[:, 0:DC * JG],
                              in_=w3d(wvT).bitcast(FP32R))
    for jc in range(2):
        nc.scalar.dma_start(out=owT_sb[:, jc * D:(jc + 1) * D],
                            in_=owT[jc * 128:(jc + 1) * 128, :])
    # broadcast bv to all partitions: [1,256] x K=1 ones -> PSUM -> SBUF
    bv_ps = ps.tile((128, JG), FP32, tag="lg01", bufs=1, name="bv_ps")
    nc.tensor.matmul(bv_ps, ones1_sb, bv_row, start=True, stop=True)
    nc.vector.tensor_copy(bv_bc, bv_ps)

    # PSUM tags: av0(1) + av1(1) + lg01(4) + op(1) + vps(1) = 8 banks.
    def proj_chain(w_sb, jc, st, b_sb, dst_sb, tag):
        pp = ps.tile((128, 512), FP32, tag=tag, bufs=1, name=f"pr{tag}")
        for dc in range(DC):
            nc.tensor.matmul(
                pp, w_sb[:, dc * JG + jc * 128: dc * JG + (jc + 1) * 128],
                xT_sb[:, dc * S + st * 512: dc * S + (st + 1) * 512],
                start=(dc == 0), stop=(dc == DC - 1))
        nc.vector.tensor_scalar_add(
            out=dst_sb[:, jc * S + st * 512: jc * S + (st + 1) * 512],
            in0=pp, scalar1=b_sb[:, jc:jc + 1])

    def v_chunk(sc):
        pv = ps.tile((128, JG), FP32, tag="vps", bufs=1, name="vps")
        for dc in range(DC):
            nc.tensor.matmul(
                pv, xT_sb[:, dc * S + sc * 128: dc * S + (sc + 1) * 128],
                wvT_sb[:, dc * JG:(dc + 1) * JG],
                start=(dc == 0), stop=(dc == DC - 1))
        base = sc * 4 * VW
        for a in range(4):
            nc.vector.tensor_add(
                out=Vaug_sb[:, base + a * VW: base + a * VW + HD],
                in0=pv[:, a * HD:(a + 1) * HD],
                in1=bv_bc[:, a * HD:(a + 1) * HD])

    # K jc0 (4 parallel chains) + Q jc0-st0 (5th chain, on the lg01 slot),
    # all dc-outer so the 5 matmuls per dc chase the x DMA chunks ->
    # everything block (qt0, pair0) needs finishes ~one matmul after the
    # last x chunk lands. The K st-chunks complete in query order,
    # matching the kc ranges the block consumes.
    kps = [ps.tile((128, 512), FP32, tag=t, bufs=1, name=f"k{t}")
           for t in ("av0", "av1", "op", "vps")]
    qp = ps.tile((128, 512), FP32, tag="lg01", bufs=1, name="q00")
    for dc in range(DC):
        for st in range(QT_TILES):
            nc.tensor.matmul(
                kps[st], wkT_sb[:, dc * JG: dc * JG + 128],
                xT_sb[:, dc * S + st * 512: dc * S + (st + 1) * 512],
                start=(dc == 0), stop=(dc == DC - 1))
        nc.tensor.matmul(
            qp, wqT_sb[:, dc * JG: dc * JG + 128],
            xT_sb[:, dc * S: dc * S + 512],
            start=(dc == 0), stop=(dc == DC - 1))
    for st in range(QT_TILES):
        nc.vector.tensor_scalar_add(
            out=KT_sb[:, st * 512:(st + 1) * 512],
            in0=kps[st], scalar1=bk_sb[:, 0:1])
    nc.vector.tensor_scalar_add(out=QT_sb[:, 0:512], in0=qp,
                                scalar1=bq_sb[:, 0:1])

    # Side-work drained into the attention blocks' PE slack: the remaining
    # projection chains (op tag) and V chunks (vps tag).
    # pop order: 6 in block 0 (kp2-7), then one per block at kp3 -- each
    # chain must land at least one block before its consumer reads it
    # (block index of Q[jc,st]'s consumer = 2*st + jc).
    side = []
    for st in range(QT_TILES):
        side.append(lambda st=st: proj_chain(wkT_sb, 1, st, bk_sb, KT_sb,
                                             "op"))
    side.append(lambda: proj_chain(wqT_sb, 1, 0, bq_sb, QT_sb, "op"))
    for st in range(1, QT_TILES):
        side.append(lambda st=st: proj_chain(wqT_sb, 0, st, bq_sb, QT_sb,
                                             "op"))
        side.append(lambda st=st: proj_chain(wqT_sb, 1, st, bq_sb, QT_sb,
                                             "op"))
    vleft = list(range(KC))

    def emit_outproj_tile(st, it):
        # alternate between the op and (post-block-0 idle) vps banks so
        # consecutive out-proj tiles don't serialize on one PSUM buffer.
        pp = ps.tile((128, 512), FP32, tag=("op" if it == 0 else "vps"),
                     bufs=1, name="op")
        for jc in range(2):
            nc.tensor.matmul(
                pp,
                attn_outT_sb[:, jc * S + st * 128: jc * S + st * 128 + 128],
                owT_sb[:, jc * D + it * 512: jc * D + (it + 1) * 512],
                start=(jc == 0), stop=(jc == 1))
        ost = p.tile((128, 512), BF16, tag="ost", bufs=4, name="ost")
        nc.vector.tensor_copy(ost, pp)
        nc.sync.dma_start(
            out=out[st * 128:(st + 1) * 128, it * 512:(it + 1) * 512],
            in_=ost)

    def emit_outproj_st(st):
        emit_outproj_tile(st, 0)
        emit_outproj_tile(st, 1)

    def make_normalize(av0, av1, qt, pair):
        # normalize, deferred into the next block so the bcast matmul's
        # dependency chain (DVE copies -> DMA scatter) resolves before it
        # reaches the head of the in-order PE queue. Stage 1 frees the av
        # banks; stage 2 does denominators + muls.
        def stage1():
            raw0 = p.tile((HD, 512), FP32, tag="raw0", bufs=2, name="raw0")
            nc.vector.tensor_copy(raw0, av0[0:HD, :])
            raw1 = p.tile((128, 512), FP32, tag="raw1", bufs=2, name="raw1")
            nc.vector.tensor_copy(raw1[HD:128, :], av1[0:HD, :])
            dstage = p.tile((1, 1024), FP32R, tag="dstage", bufs=2,
                            name="dstage")
            nc.vector.tensor_copy(dstage[:, 0:512], av0[HD:HD + 1, :])
            nc.vector.tensor_copy(dstage[:, 512:1024], av1[HD:HD + 1, :])
            drows = p.tile((2, 512), FP32R, tag="drow", bufs=2, name="drow")
            nc.sync.dma_start(out=drows, in_=dstage)

            def stage2():
                bc = ps.tile((128, 512), FP32, tag="op", bufs=1, name="bc")
                nc.tensor.matmul(bc, ones2_sb, drows, start=True, stop=True)
                rcs = p.tile((128, 512), FP32, tag="rcs", bufs=2, name="rcs")
                nc.vector.reciprocal_approx_fast(rcs, bc)
                base = pair * S + qt * 512
                nc.vector.tensor_mul(
                    out=attn_outT_sb[0:HD, base:base + 512],
                    in0=raw0[0:HD, :], in1=rcs[0:HD, :])
                nc.vector.tensor_mul(
                    out=attn_outT_sb[HD:128, base:base + 512],
                    in0=raw1[HD:128, :], in1=rcs[HD:128, :])
            return stage2
        return stage1

    pending_norm = None
    op_work = []
    for qt in range(QT_TILES):
        for pair in range(2):
            h0 = 2 * pair
            first_block = (qt == 0 and pair == 0)
            av0 = ps.tile((128, 512), FP32, tag="av0", bufs=1, name="av0")
            av1 = ps.tile((128, 512), FP32, tag="av1", bufs=1, name="av1")
            qcol = pair * S + qt * 512

            def emit_avs(at, kc0, kc1):
                for h, avp in ((h0, av0), (h0 + 1, av1)):
                    for i, kc in ((0, kc0), (1, kc1)):
                        nc.tensor.matmul(
                            avp[0:VW, :],
                            Vaug_sb[:, kc * 4 * VW + h * VW:
                                    kc * 4 * VW + (h + 1) * VW],
                            at[:, i * 1024 + (h % 2) * 512:
                               i * 1024 + (h % 2) * 512 + 512],
                            start=(kc == 0), stop=(kc == KC - 1))

            prev = None
            for kp in range(KC // 2):
                kc0, kc1 = 2 * kp, 2 * kp + 1
                lg = ps.tile((128, 2048), FP32, tag="lg01", bufs=1,
                             name="lg")
                # bank layout: [h0@kc0 | h1@kc0 | h0@kc1 | h1@kc1] -- each
                # kc's head-pair is adjacent (row-packed concurrent on the
                # PE) AND lives in one exp's bank-half, so the next kp's
                # first pair only WARs the first exp and issues under the
                # second -> the Scalar engine stays back-to-back.
                for i, kc in ((0, kc0), (1, kc1)):
                    kcol = pair * S + kc * 128
                    nc.tensor.matmul(
                        lg[:, i * 1024: i * 1024 + 512],
                        KT_sb[0:64, kcol:kcol + 128],
                        QT_sb[0:64, qcol:qcol + 512],
                        start=True, stop=True, tile_position=(0, 0))
                    nc.tensor.matmul(
                        lg[:, i * 1024 + 512: i * 1024 + 1024],
                        KT_sb[64:128, kcol:kcol + 128],
                        QT_sb[64:128, qcol:qcol + 512],
                        start=True, stop=True, tile_position=(64, 0))
                # The previous kp's AV matmuls are emitted after this kp's
                # exps so they fill the PE during the exp shadow instead of
                # serializing exp -> av -> next-lg.
                at = p.tile((128, 2048), BF16, tag="at", bufs=3, name="at")
                nc.scalar.activation(at[:, 0:1024], lg[:, 0:1024], EXP)
                nc.scalar.activation(at[:, 1024:2048], lg[:, 1024:2048], EXP)
                # stage1 must be emitted before this block's first AV
                # matmuls (kp==1) -- it reads the av banks they overwrite.
                if pending_norm is not None and kp == 0:
                    pending_norm = pending_norm()
                if prev is not None:
                    emit_avs(*prev)
                prev = (at, kc0, kc1)
                # PE filler, emitted after the exps so it runs in the exp
                # shadow: the deferred normalize pieces, one projection
                # side-chain, or one out-proj tile per kc-pair (a batch at
                # a block boundary would bubble the Scalar engine).
                if first_block:
                    # V chunks just-in-time, in the exp shadow: chunk sc
                    # feeds this block's AV matmuls at kp = sc//2, which
                    # execute one kp later; later blocks reuse Vaug.
                    v_chunk(vleft.pop(0))
                    v_chunk(vleft.pop(0))
                    if kp >= 2 and side:
                        side.pop(0)()
                if not first_block:
                    if pending_norm is not None and kp == 2:
                        pending_norm()
                        pending_norm = None
                        if pair == 0 and qt > 0:
                            # qt-1's attn_outT is now fully normalized
                            op_work += [
                                (lambda st=st, it=it:
                                 emit_outproj_tile(st, it))
                                for st in range(4 * (qt - 1), 4 * qt)
                                for it in range(2)]
                    elif kp == 3 and side:
                        side.pop(0)()
                    elif kp >= 3 and op_work:
                        op_work.pop(0)()
            emit_avs(*prev)
            pending_norm = make_normalize(av0, av1, qt, pair)
    # tail: the last block's normalize, chunked in 128-column pieces with
    # the final out-proj tiles interleaved (st 12+c only needs chunk c of
    # the final muls), so the serial tail chain is as short as possible.
    while op_work:
        op_work.pop(0)()
    raw0 = p.tile((HD, 512), FP32, tag="raw0", bufs=2, name="raw0")
    nc.vector.tensor_copy(raw0, av0[0:HD, :])
    raw1 = p.tile((128, 512), FP32, tag="raw1", bufs=2, name="raw1")
    nc.vector.tensor_copy(raw1[HD:128, :], av1[0:HD, :])
    dstage = p.tile((1, 1024), FP32R, tag="dstage", bufs=2, name="dstage")
    nc.vector.tensor_copy(dstage[:, 0:512], av0[HD:HD + 1, :])
    nc.vector.tensor_copy(dstage[:, 512:1024], av1[HD:HD + 1, :])
    drows = p.tile((2, 512), FP32R, tag="drow", bufs=2, name="drow")
    nc.sync.dma_start(out=drows, in_=dstage)
    bc = ps.tile((128, 512), FP32, tag="op", bufs=1, name="bc")
    nc.tensor.matmul(bc, ones2_sb, drows, start=True, stop=True)
    rcs = p.tile((128, 512), FP32, tag="rcs", bufs=2, name="rcs")
    base = S + (QT_TILES - 1) * 512
    for c in range(4):
        lo, hi = c * 128, (c + 1) * 128
        nc.vector.reciprocal_approx_fast(rcs[:, lo:hi], bc[:, lo:hi])
        nc.vector.tensor_mul(out=attn_outT_sb[0:HD, base + lo:base + hi],
                             in0=raw0[0:HD, lo:hi], in1=rcs[0:HD, lo:hi])
        nc.vector.tensor_mul(out=attn_outT_sb[HD:128, base + lo:base + hi],
                             in0=raw1[HD:128, lo:hi], in1=rcs[HD:128, lo:hi])
        emit_outproj_tile(4 * (QT_TILES - 1) + c, 0)
        emit_outproj_tile(4 * (QT_TILES - 1) + c, 1)


_NC = None


def _build_nc():
    global _NC
    if _NC is not None:
        return _NC
    nc = bacc.Bacc("TRN2", target_bir_lowering=False, debug=False,
                   num_devices=NCORES)
    xT = nc.dram_tensor("xT", [D, S], FP32, kind="ExternalInput").ap()
    wqT = nc.dram_tensor("wqT", [D, JG], FP32, kind="ExternalInput").ap()
    wkT = nc.dram_tensor("wkT", [D, JG], FP32, kind="ExternalInput").ap()
    wvT = nc.dram_tensor("wvT", [D, JG], FP32, kind="ExternalInput").ap()
    bq = nc.dram_tensor("bq", [JG], FP32, kind="ExternalInput").ap()
    bk = nc.dram_tensor("bk", [JG], FP32, kind="ExternalInput").ap()
    bv = nc.dram_tensor("bv", [JG], FP32, kind="ExternalInput").ap()
    owT = nc.dram_tensor("owT", [JG, D], BF16, kind="ExternalInput").ap()
    out = nc.dram_tensor("out", [S, D], BF16, kind="ExternalOutput").ap()
    with tile.TileContext(nc) as tc:
        mha_core_kernel(tc, out, xT, wqT, wkT, wvT, bq, bk, bv, owT)
    nc.compile()
    _NC = nc
    return nc


def _in_maps(x, kqv_w, kqv_b, out_w):
    maps = []
    for c in range(NCORES):
        b, g = divmod(c, 4)
        sl = slice(g * JG, (g + 1) * JG)
        maps.append({
            "xT": np.ascontiguousarray(x[b].T),
            "wqT": np.ascontiguousarray(kqv_w[0 * D:1 * D][sl].T),
            "wkT": np.ascontiguousarray(kqv_w[1 * D:2 * D][sl].T),
            "wvT": np.ascontiguousarray(kqv_w[2 * D:3 * D][sl].T),
            "bq": np.ascontiguousarray(kqv_b[0 * D:1 * D][sl]),
            "bk": np.ascontiguousarray(kqv_b[1 * D:2 * D][sl]),
            "bv": np.ascontiguousarray(kqv_b[2 * D:3 * D][sl]),
            "owT": np.ascontiguousarray(out_w[:, sl].T).astype(
                ml_dtypes.bfloat16),
        })
    return maps


def run_spmd(x, kqv_w, kqv_b, out_w, out_b, trace=False, tmpdir=None):
    nc = _build_nc()
    res = run_bass_kernel_spmd(nc, _in_maps(x, kqv_w, kqv_b, out_w),
                               list(range(NCORES)), tmpdir=tmpdir, trace=trace)
    parts = [np.asarray(res.results[c]["out"]).astype(np.float32)
             for c in range(NCORES)]
    full = np.stack([
        parts[4 * b] + parts[4 * b + 1] + parts[4 * b + 2] + parts[4 * b + 3]
        + out_b[None, :].astype(np.float32)
        for b in range(B)
    ])
    return full, res


def kernel(**inputs):
    x = np.asarray(inputs["x"], dtype=np.float32)
    kqv_w = np.asarray(inputs["kqv_w"], dtype=np.float32)
    kqv_b = np.asarray(inputs["kqv_b"], dtype=np.float32)
    out_w = np.asarray(inputs["out_w"], dtype=np.float32)
    out_b = np.asarray(inputs["out_b"], dtype=np.float32)
    full, _ = run_spmd(x, kqv_w, kqv_b, out_w, out_b)
    return full


# revision 31
# speedup vs baseline: 1.0183x; 1.0183x over previous
"""MultiHeadAttention (B=2, S=2048, D=1024, H=16) on 8 TRN2 NeuronCores.

Sharding: core c -> batch b = c//4, head-group g = c%4 (4 heads = 256 channels).
Each core computes its 4 heads' attention for its batch plus the partial
out-projection (out_w columns for its channel group); host sums the 4 partials
per batch and adds out_b.

v3 (from the 359us v2):
  - both heads' logits for a key-chunk pair land in ONE [128,2048] 4-bank
    PSUM tile -> a single N=2048 ACTIVATE per kc-pair (25% less Scalar time
    than 2xN=1024, half the ACT semaphore overhead), and the four logits
    matmuls share identical deps so the scheduler keeps the row-packed
    (tile_position 0/64) pairs adjacent -> they run concurrently on the PE
    (v2's per-head exp chains made the scheduler serialize the pairs).
  - phase A is folded into phase B: K(jc0)+Q(jc0,st0) projections run first
    (DMA ring reordered weights-first), attention block 0 starts ~30us in,
    and the remaining K(jc1)/Q chains + all V chunks trickle through the
    PSUM 'op'/'vps' tags inside the attention blocks' PE slack, instead of
    an 82us serial cold-clock phase A.
  - Q/K path fp32r (logits precision), V/exp-out/AV/attn-out/out-proj bf16;
    softmax denominators via the Vaug ones-column + K=2 block-ones broadcast
    matmul + reciprocal_approx_fast.
"""

import os
import sys

import numpy as np

for _p in ("/opt/trn_rl_repo",):
    if os.path.isdir(_p) and _p not in sys.path:
        sys.path.insert(0, _p)

from contextlib import ExitStack

import ml_dtypes

import concourse.bass as bass
import concourse.tile as tile
from concourse import bacc, mybir
from concourse._compat import with_exitstack
from concourse.bass_utils import run_bass_kernel_spmd

B, S, D = 2, 2048, 1024
H = 16
HD = 64
NCORES = 8
JG = 256          # channels per core (4 heads)
DC = D // 128     # 8 contraction chunks
QT_TILES = 4      # 4 x 512 query tiles
KC = S // 128     # 16 key chunks
VW = 65           # V columns per head incl. ones column
FP32 = mybir.dt.float32
FP32R = mybir.dt.float32r
BF16 = mybir.dt.bfloat16
EXP = mybir.ActivationFunctionType.Exp


@with_exitstack
def mha_core_kernel(ctx: ExitStack, tc: tile.TileContext,
                    out, xT, wqT, wkT, wvT, bq, bk, bv, owT):
    nc = tc.nc
    ctx.enter_context(nc.allow_low_precision("bf16 V/AV/out-proj path"))

    p = ctx.enter_context(tc.tile_pool(name="p", bufs=1))
    ps = ctx.enter_context(tc.tile_pool(name="ps", bufs=1, space="PSUM"))

    # QT/KT in bf16: the logits matmuls' SBUF streams are the largest PE
    # power draw (HAM duty), and bf16 Q/K costs ~9e-3 rel err (numpy-
    # verified 'fp32_x_qkvw' config) vs the 2e-2 gate.
    QT_sb = p.tile((128, 2 * S), BF16)
    KT_sb = p.tile((128, 2 * S), BF16)
    Vaug_sb = p.tile((128, KC * 4 * VW), BF16)
    attn_outT_sb = p.tile((128, 2 * S), BF16)
    owT_sb = p.tile((128, 2 * D), BF16)
    ones2_sb = p.tile((2, 128), FP32R)
    xT_sb = p.tile((128, DC * S), FP32R)
    wqT_sb = p.tile((128, DC * JG), FP32R)
    wkT_sb = p.tile((128, DC * JG), FP32R)
    wvT_sb = p.tile((128, DC * JG), FP32R)
    bq_sb = p.tile((128, 2), FP32)
    bk_sb = p.tile((128, 2), FP32)
    bv_bc = p.tile((128, JG), FP32)
    bv_row = p.tile((1, JG), FP32R)
    ones1_sb = p.tile((1, 128), FP32R)
    ones_f32 = p.tile((128, 64), FP32)
    ones2_f32 = p.tile((2, 128), FP32)

    # Stage fp32 ones and DVE-copy (cast) into the bf16 ones columns of
    # Vaug + the fp32r 2-row block-ones used for the denominator bcast.
    nc.vector.memset(ones_f32, 1.0)
    nc.vector.tensor_copy(Vaug_sb[:, HD::VW], ones_f32)
    nc.vector.memset(ones2_f32, 0.0)
    nc.vector.memset(ones2_f32[0:1, 0:64], 1.0)
    # DVE memset can't start at partition 1; DMA-copy the ones block.
    nc.sync.dma_start(out=ones2_f32[1:2, 64:128], in_=ones2_f32[0:1, 0:64])
    nc.vector.tensor_copy(ones2_sb, ones2_f32)
    nc.vector.tensor_copy(ones1_sb[:, 0:64], ones_f32[0:1, 0:64])
    nc.vector.tensor_copy(ones1_sb[:, 64:128], ones_f32[0:1, 0:64])

    # DMA: everything on the sync HWDGE ring (the gpsimd software ring
    # measured only ~50GB/s -- putting wk there gated the whole kernel).
    # The weights are single 3D transfers ([p][dc][col] so each partition
    # line is a contiguous 8KB instead of 8x1KB strided chunks). Order:
    # wk+wq before x (the K/Q chains chase the x chunks), wv mid-x (first
    # V chunk isn't needed until the first attention block), owT last.
    def w3d(w):
        return bass.AP(tensor=w.tensor, offset=w.offset,
                       ap=[[JG, 128], [128 * JG, DC], [1, JG]])

    nc.sync.dma_start(out=wkT_sb[:, 0:DC * JG], in_=w3d(wkT).bitcast(FP32R))
    nc.sync.dma_start(out=wqT_sb[:, 0:DC * JG], in_=w3d(wqT).bitcast(FP32R))
    for jc in range(2):
        nc.sync.dma_start(out=bk_sb[:, jc:jc + 1],
                          in_=bk[jc * 128:(jc + 1) * 128])
        nc.sync.dma_start(out=bq_sb[:, jc:jc + 1],
                          in_=bq[jc * 128:(jc + 1) * 128])
    nc.sync.dma_start(out=bv_row, in_=bv.bitcast(FP32R))
    for dc in range(4):
        nc.sync.dma_start(out=xT_sb[:, dc * S:(dc + 1) * S],
                          in_=xT[dc * 128:(dc + 1) * 128, :].bitcast(FP32R))
    nc.sync.dma_start(out=wvT_sb[:, 0:DC * JG], in_=w3d(wvT).bitcast(FP32R))
    for dc in range(4, DC):
        nc.sync.dma_start(out=xT_sb[:, dc * S:(dc + 1) * S],
                          in_=xT[dc * 128:(dc + 1) * 128, :].bitcast(FP32R))
    for jc in range(2):
        nc.sync.dma_start(out=owT_sb[:, jc * D:(jc + 1) * D],
                          in_=owT[jc * 128:(jc + 1) * 128, :])
    # broadcast bv to all partitions: [1,256] x K=1 ones -> PSUM -> SBUF
    bv_ps = ps.tile((128, JG), FP32, tag="lg01", bufs=1, name="bv_ps")
    nc.tensor.matmul(bv_ps, ones1_sb, bv_row, start=True, stop=True)
    nc.vector.tensor_copy(bv_bc, bv_ps)

    # PSUM tags: av0(1) + av1(1) + lg01(4) + op(1) + vps(1) = 8 banks.
    def proj_chain(w_sb, jc, st, b_sb, dst_sb, tag):
        pp = ps.tile((128, 512), FP32, tag=tag, bufs=1, name=f"pr{tag}")
        for dc in range(DC):
            nc.tensor.matmul(
                pp, w_sb[:, dc * JG + jc * 128: dc * JG + (jc + 1) * 128],
                xT_sb[:, dc * S + st * 512: dc * S + (st + 1) * 512],
                start=(dc == 0), stop=(dc == DC - 1))
        nc.vector.tensor_scalar_add(
            out=dst_sb[:, jc * S + st * 512: jc * S + (st + 1) * 512],
            in0=pp, scalar1=b_sb[:, jc:jc + 1])

    def v_chunk(sc):
        pv = ps.tile((128, JG), FP32, tag="vps", bufs=1, name="vps")
        for dc in range(DC):
            nc.tensor.matmul(
                pv, xT_sb[:, dc * S + sc * 128: dc * S + (sc + 1) * 128],
                wvT_sb[:, dc * JG:(dc + 1) * JG],
                start=(dc == 0), stop=(dc == DC - 1))
        base = sc * 4 * VW
        for a in range(4):
            nc.vector.tensor_add(
                out=Vaug_sb[:, base + a * VW: base + a * VW + HD],
                in0=pv[:, a * HD:(a + 1) * HD],
                in1=bv_bc[:, a * HD:(a + 1) * HD])

    # K jc0 (4 parallel chains) + Q jc0-st0 (5th chain, on the lg01 slot),
    # all dc-outer so the 5 matmuls per dc chase the x DMA chunks ->
    # everything block (qt0, pair0) needs finishes ~one matmul after the
    # last x chunk lands. The K st-chunks complete in query order,
    # matching the kc ranges the block consumes.
    kps = [ps.tile((128, 512), FP32, tag=t, bufs=1, name=f"k{t}")
           for t in ("av0", "av1", "op", "vps")]
    qp = ps.tile((128, 512), FP32, tag="lg01", bufs=1, name="q00")
    for dc in range(DC):
        for st in range(QT_TILES):
            nc.tensor.matmul(
                kps[st], wkT_sb[:, dc * JG: dc * JG + 128],
                xT_sb[:, dc * S + st * 512: dc * S + (st + 1) * 512],
                start=(dc == 0), stop=(dc == DC - 1))
        nc.tensor.matmul(
            qp, wqT_sb[:, dc * JG: dc * JG + 128],
            xT_sb[:, dc * S: dc * S + 512],
            start=(dc == 0), stop=(dc == DC - 1))
    for st in range(QT_TILES):
        nc.vector.tensor_scalar_add(
            out=KT_sb[:, st * 512:(st + 1) * 512],
            in0=kps[st], scalar1=bk_sb[:, 0:1])
    nc.vector.tensor_scalar_add(out=QT_sb[:, 0:512], in0=qp,
                                scalar1=bq_sb[:, 0:1])

    # Side-work drained into the attention blocks' PE slack: the remaining
    # projection chains (op tag) and V chunks (vps tag).
    # pop order: 6 in block 0 (kp2-7), then one per block at kp3 -- each
    # chain must land at least one block before its consumer reads it
    # (block index of Q[jc,st]'s consumer = 2*st + jc).
    side = []
    for st in range(QT_TILES):
        side.append(lambda st=st: proj_chain(wkT_sb, 1, st, bk_sb, KT_sb,
                                             "op"))
    side.append(lambda: proj_chain(wqT_sb, 1, 0, bq_sb, QT_sb, "op"))
    for st in range(1, QT_TILES):
        side.append(lambda st=st: proj_chain(wqT_sb, 0, st, bq_sb, QT_sb,
                                             "op"))
        side.append(lambda st=st: proj_chain(wqT_sb, 1, st, bq_sb, QT_sb,
                                             "op"))
    vleft = list(range(KC))

    def emit_outproj_tile(st, it):
        # alternate between the op and (post-block-0 idle) vps banks so
        # consecutive out-proj tiles don't serialize on one PSUM buffer.
        pp = ps.tile((128, 512), FP32, tag=("op" if it == 0 else "vps"),
                     bufs=1, name="op")
        for jc in range(2):
            nc.tensor.matmul(
                pp,
                attn_outT_sb[:, jc * S + st * 128: jc * S + st * 128 + 128],
                owT_sb[:, jc * D + it * 512: jc * D + (it + 1) * 512],
                start=(jc == 0), stop=(jc == 1))
        ost = p.tile((128, 512), BF16, tag="ost", bufs=4, name="ost")
        nc.vector.tensor_copy(ost, pp)
        nc.sync.dma_start(
            out=out[st * 128:(st + 1) * 128, it * 512:(it + 1) * 512],
            in_=ost)

    def emit_outproj_st(st):
        emit_outproj_tile(st, 0)
        emit_outproj_tile(st, 1)

    def make_normalize(av0, av1, qt, pair):
        # normalize, deferred into the next block so the bcast matmul's
        # dependency chain (DVE copies -> DMA scatter) resolves before it
        # reaches the head of the in-order PE queue. Stage 1 frees the av
        # banks; stage 2 does denominators + muls.
        def stage1():
            raw0 = p.tile((HD, 512), FP32, tag="raw0", bufs=2, name="raw0")
            nc.vector.tensor_copy(raw0, av0[0:HD, :])
            raw1 = p.tile((128, 512), FP32, tag="raw1", bufs=2, name="raw1")
            nc.vector.tensor_copy(raw1[HD:128, :], av1[0:HD, :])
            dstage = p.tile((1, 1024), FP32R, tag="dstage", bufs=2,
                            name="dstage")
            nc.vector.tensor_copy(dstage[:, 0:512], av0[HD:HD + 1, :])
            nc.vector.tensor_copy(dstage[:, 512:1024], av1[HD:HD + 1, :])
            drows = p.tile((2, 512), FP32R, tag="drow", bufs=2, name="drow")
            nc.sync.dma_start(out=drows, in_=dstage)

            def stage2():
                bc = ps.tile((128, 512), FP32, tag="op", bufs=1, name="bc")
                nc.tensor.matmul(bc, ones2_sb, drows, start=True, stop=True)
                rcs = p.tile((128, 512), FP32, tag="rcs", bufs=2, name="rcs")
                nc.vector.reciprocal_approx_fast(rcs, bc)
                base = pair * S + qt * 512
                nc.vector.tensor_mul(
                    out=attn_outT_sb[0:HD, base:base + 512],
                    in0=raw0[0:HD, :], in1=rcs[0:HD, :])
                nc.vector.tensor_mul(
                    out=attn_outT_sb[HD:128, base:base + 512],
                    in0=raw1[HD:128, :], in1=rcs[HD:128, :])
            return stage2
        return stage1

    pending_norm = None
    op_work = []
    for qt in range(QT_TILES):
        for pair in range(2):
            h0 = 2 * pair
            first_block = (qt == 0 and pair == 0)
            av0 = ps.tile((128, 512), FP32, tag="av0", bufs=1, name="av0")
            av1 = ps.tile((128, 512), FP32, tag="av1", bufs=1, name="av1")
            qcol = pair * S + qt * 512

            def emit_avs(at, kc0, kc1):
                for h, avp in ((h0, av0), (h0 + 1, av1)):
                    for i, kc in ((0, kc0), (1, kc1)):
                        nc.tensor.matmul(
                            avp[0:VW, :],
                            Vaug_sb[:, kc * 4 * VW + h * VW:
                                    kc * 4 * VW + (h + 1) * VW],
                            at[:, i * 1024 + (h % 2) * 512:
                               i * 1024 + (h % 2) * 512 + 512],
                            start=(kc == 0), stop=(kc == KC - 1))

            prev = None
            for kp in range(KC // 2):
                kc0, kc1 = 2 * kp, 2 * kp + 1
                lg = ps.tile((128, 2048), FP32, tag="lg01", bufs=1,
                             name="lg")
                # bank layout: [h0@kc0 | h1@kc0 | h0@kc1 | h1@kc1] -- each
                # kc's head-pair is adjacent (row-packed concurrent on the
                # PE) AND lives in one exp's bank-half, so the next kp's
                # first pair only WARs the first exp and issues under the
                # second -> the Scalar engine stays back-to-back.
                for i, kc in ((0, kc0), (1, kc1)):
                    kcol = pair * S + kc * 128
                    nc.tensor.matmul(
                        lg[:, i * 1024: i * 1024 + 512],
                        KT_sb[0:64, kcol:kcol + 128],
                        QT_sb[0:64, qcol:qcol + 512],
                        start=True, stop=True, tile_position=(0, 0))
                    nc.tensor.matmul(
                        lg[:, i * 1024 + 512: i * 1024 + 1024],
                        KT_sb[64:128, kcol:kcol + 128],
                        QT_sb[64:128, qcol:qcol + 512],
                        start=True, stop=True, tile_position=(64, 0))
                # The previous kp's AV matmuls are emitted after this kp's
                # exps so they fill the PE during the exp shadow instead of
                # serializing exp -> av -> next-lg.
                at = p.tile((128, 2048), BF16, tag="at", bufs=3, name="at")
                nc.scalar.activation(at[:, 0:1024], lg[:, 0:1024], EXP)
                nc.scalar.activation(at[:, 1024:2048], lg[:, 1024:2048], EXP)
                # stage1 must be emitted before this block's first AV
                # matmuls (kp==1) -- it reads the av banks they overwrite.
                if pending_norm is not None and kp == 0:
                    pending_norm = pending_norm()
                if prev is not None:
                    emit_avs(*prev)
                prev = (at, kc0, kc1)
                # PE filler, emitted after the exps so it runs in the exp
                # shadow: the deferred normalize pieces, one projection
                # side-chain, or one out-proj tile per kc-pair (a batch at
                # a block boundary would bubble the Scalar engine).
                if first_block:
                    # V chunks just-in-time, in the exp shadow: chunk sc
                    # feeds this block's AV matmuls at kp = sc//2, which
                    # execute one kp later; later blocks reuse Vaug.
                    v_chunk(vleft.pop(0))
                    v_chunk(vleft.pop(0))
                    if kp >= 2 and side:
                        side.pop(0)()
                if not first_block:
                    if pending_norm is not None and kp == 2:
                        pending_norm()
                        pending_norm = None
                        if pair == 0 and qt > 0:
                            # qt-1's attn_outT is now fully normalized
                            op_work += [
                                (lambda st=st, it=it:
                                 emit_outproj_tile(st, it))
                                for st in range(4 * (qt - 1), 4 * qt)
                                for it in range(2)]
                    elif kp == 3 and side:
                        side.pop(0)()
                    elif kp >= 3 and op_work:
                        op_work.pop(0)()
            emit_avs(*prev)
            pending_norm = make_normalize(av0, av1, qt, pair)
    # tail: the last block's normalize, chunked in 128-column pieces with
    # the final out-proj tiles interleaved (st 12+c only needs chunk c of
    # the final muls), so the serial tail chain is as short as possible.
    while op_work:
        op_work.pop(0)()
    raw0 = p.tile((HD, 512), FP32, tag="raw0", bufs=2, name="raw0")
    nc.vector.tensor_copy(raw0, av0[0:HD, :])
    raw1 = p.tile((128, 512), FP32, tag="raw1", bufs=2, name="raw1")
    nc.vector.tensor_copy(raw1[HD:128, :], av1[0:HD, :])
    dstage = p.tile((1, 1024), FP32R, tag="dstage", bufs=2, name="dstage")
    nc.vector.tensor_copy(dstage[:, 0:512], av0[HD:HD + 1, :])
    nc.vector.tensor_copy(dstage[:, 512:1024], av1[HD:HD + 1, :])
    drows = p.tile((2, 512), FP32R, tag="drow", bufs=2, name="drow")
    nc.sync.dma_start(out=drows, in_=dstage)
    bc = ps.tile((128, 512), FP32, tag="op", bufs=1, name="bc")
    nc.tensor.matmul(bc, ones2_sb, drows, start=True, stop=True)
    rcs = p.tile((128, 512), FP32, tag="rcs", bufs=2, name="rcs")
    base = S + (QT_TILES - 1) * 512
    for c in range(4):
        lo, hi = c * 128, (c + 1) * 128
        nc.vector.reciprocal_approx_fast(rcs[:, lo:hi], bc[:, lo:hi])
        nc.vector.tensor_mul(out=attn_outT_sb[0:HD, base + lo:base + hi],
                             in0=raw0[0:HD, lo:hi], in1=rcs[0:HD, lo:hi])
        nc.vector.tensor_mul(out=attn_outT_sb[HD:128, base + lo:base + hi],
                             in0=raw1[HD:128, lo:hi], in1=rcs[HD:128, lo:hi])
        emit_outproj_tile(4 * (QT_TILES - 1) + c, 0)
        emit_outproj_tile(4 * (QT_TILES - 1) + c, 1)


_NC = None


def _build_nc():
    global _NC
    if _NC is not None:
        return _NC
    nc = bacc.Bacc("TRN2", target_bir_lowering=False, debug=False,
                   num_devices=NCORES)
    xT = nc.dram_tensor("xT", [D, S], FP32, kind="ExternalInput").ap()
    wqT = nc.dram_tensor("wqT", [D, JG], FP32, kind="ExternalInput").ap()
    wkT = nc.dram_tensor("wkT", [D, JG], FP32, kind="ExternalInput").ap()
    wvT = nc.dram_tensor("wvT", [D, JG], FP32, kind="ExternalInput").ap()
    bq = nc.dram_tensor("bq", [JG], FP32, kind="ExternalInput").ap()
    bk = nc.dram_tensor("bk", [JG], FP32, kind="ExternalInput").ap()
    bv = nc.dram_tensor("bv", [JG], FP32, kind="ExternalInput").ap()
    owT = nc.dram_tensor("owT", [JG, D], BF16, kind="ExternalInput").ap()
    out = nc.dram_tensor("out", [S, D], BF16, kind="ExternalOutput").ap()
    with tile.TileContext(nc) as tc:
        mha_core_kernel(tc, out, xT, wqT, wkT, wvT, bq, bk, bv, owT)
    nc.compile()
    _NC = nc
    return nc


def _in_maps(x, kqv_w, kqv_b, out_w):
    maps = []
    for c in range(NCORES):
        b, g = divmod(c, 4)
        sl = slice(g * JG, (g + 1) * JG)
        maps.append({
            "xT": np.ascontiguousarray(x[b].T),
            "wqT": np.ascontiguousarray(kqv_w[0 * D:1 * D][sl].T),
            "wkT": np.ascontiguousarray(kqv_w[1 * D:2 * D][sl].T),
            "wvT": np.ascontiguousarray(kqv_w[2 * D:3 * D][sl].T),
            "bq": np.ascontiguousarray(kqv_b[0 * D:1 * D][sl]),
            "bk": np.ascontiguousarray(kqv_b[1 * D:2 * D][sl]),
            "bv": np.ascontiguousarray(kqv_b[2 * D:3 * D][sl]),
            "owT": np.ascontiguousarray(out_w[:, sl].T).astype(
                ml_dtypes.bfloat16),
        })
    return maps


def run_spmd(x, kqv_w, kqv_b, out_w, out_b, trace=False, tmpdir=None):
    nc = _build_nc()
    res = run_bass_kernel_spmd(nc, _in_maps(x, kqv_w, kqv_b, out_w),
                               list(range(NCORES)), tmpdir=tmpdir, trace=trace)
    parts = [np.asarray(res.results[c]["out"]).astype(np.float32)
             for c in range(NCORES)]
    full = np.stack([
        parts[4 * b] + parts[4 * b + 1] + parts[4 * b + 2] + parts[4 * b + 3]
        + out_b[None, :].astype(np.float32)
        for b in range(B)
    ])
    return full, res


def kernel(**inputs):
    x = np.asarray(inputs["x"], dtype=np.float32)
    kqv_w = np.asarray(inputs["kqv_w"], dtype=np.float32)
    kqv_b = np.asarray(inputs["kqv_b"], dtype=np.float32)
    out_w = np.asarray(inputs["out_w"], dtype=np.float32)
    out_b = np.asarray(inputs["out_b"], dtype=np.float32)
    full, _ = run_spmd(x, kqv_w, kqv_b, out_w, out_b)
    return full


# revision 32
# speedup vs baseline: 1.0327x; 1.0141x over previous
"""MultiHeadAttention (B=2, S=2048, D=1024, H=16) on 8 TRN2 NeuronCores.

Sharding: core c -> batch b = c//4, head-group g = c%4 (4 heads = 256 channels).
Each core computes its 4 heads' attention for its batch plus the partial
out-projection (out_w columns for its channel group); host sums the 4 partials
per batch and adds out_b.

v3 (from the 359us v2):
  - both heads' logits for a key-chunk pair land in ONE [128,2048] 4-bank
    PSUM tile -> a single N=2048 ACTIVATE per kc-pair (25% less Scalar time
    than 2xN=1024, half the ACT semaphore overhead), and the four logits
    matmuls share identical deps so the scheduler keeps the row-packed
    (tile_position 0/64) pairs adjacent -> they run concurrently on the PE
    (v2's per-head exp chains made the scheduler serialize the pairs).
  - phase A is folded into phase B: K(jc0)+Q(jc0,st0) projections run first
    (DMA ring reordered weights-first), attention block 0 starts ~30us in,
    and the remaining K(jc1)/Q chains + all V chunks trickle through the
    PSUM 'op'/'vps' tags inside the attention blocks' PE slack, instead of
    an 82us serial cold-clock phase A.
  - Q/K path fp32r (logits precision), V/exp-out/AV/attn-out/out-proj bf16;
    softmax denominators via the Vaug ones-column + K=2 block-ones broadcast
    matmul + reciprocal_approx_fast.
"""

import os
import sys

import numpy as np

for _p in ("/opt/trn_rl_repo",):
    if os.path.isdir(_p) and _p not in sys.path:
        sys.path.insert(0, _p)

from contextlib import ExitStack

import ml_dtypes

import concourse.bass as bass
import concourse.tile as tile
from concourse import bacc, mybir
from concourse._compat import with_exitstack
from concourse.bass_utils import run_bass_kernel_spmd

B, S, D = 2, 2048, 1024
H = 16
HD = 64
NCORES = 8
JG = 256          # channels per core (4 heads)
DC = D // 128     # 8 contraction chunks
QT_TILES = 4      # 4 x 512 query tiles
KC = S // 128     # 16 key chunks
VW = 65           # V columns per head incl. ones column
FP32 = mybir.dt.float32
FP32R = mybir.dt.float32r
BF16 = mybir.dt.bfloat16
EXP = mybir.ActivationFunctionType.Exp


@with_exitstack
def mha_core_kernel(ctx: ExitStack, tc: tile.TileContext,
                    out, xT, wqT, wkT, wvT, bq, bk, bv, owT):
    nc = tc.nc
    ctx.enter_context(nc.allow_low_precision("bf16 V/AV/out-proj path"))

    p = ctx.enter_context(tc.tile_pool(name="p", bufs=1))
    ps = ctx.enter_context(tc.tile_pool(name="ps", bufs=1, space="PSUM"))

    # QT/KT in bf16: the logits matmuls' SBUF streams are the largest PE
    # power draw (HAM duty), and bf16 Q/K costs ~9e-3 rel err (numpy-
    # verified 'fp32_x_qkvw' config) vs the 2e-2 gate.
    QT_sb = p.tile((128, 2 * S), BF16)
    KT_sb = p.tile((128, 2 * S), BF16)
    Vaug_sb = p.tile((128, KC * 4 * VW), BF16)
    attn_outT_sb = p.tile((128, 2 * S), BF16)
    owT_sb = p.tile((128, 2 * D), BF16)
    ones2_sb = p.tile((2, 128), FP32R)
    xT_sb = p.tile((128, DC * S), FP32R)
    wqT_sb = p.tile((128, DC * JG), FP32R)
    wkT_sb = p.tile((128, DC * JG), FP32R)
    wvT_sb = p.tile((128, DC * JG), FP32R)
    bq_sb = p.tile((128, 2), FP32)
    bk_sb = p.tile((128, 2), FP32)
    bv_bc = p.tile((128, JG), FP32)
    bv_row = p.tile((1, JG), FP32R)
    ones1_sb = p.tile((1, 128), FP32R)
    ones_f32 = p.tile((128, 64), FP32)
    ones2_f32 = p.tile((2, 128), FP32)

    # Stage fp32 ones and DVE-copy (cast) into the bf16 ones columns of
    # Vaug + the fp32r 2-row block-ones used for the denominator bcast.
    nc.vector.memset(ones_f32, 1.0)
    nc.vector.tensor_copy(Vaug_sb[:, HD::VW], ones_f32)
    nc.vector.memset(ones2_f32, 0.0)
    nc.vector.memset(ones2_f32[0:1, 0:64], 1.0)
    # DVE memset can't start at partition 1; DMA-copy the ones block.
    nc.sync.dma_start(out=ones2_f32[1:2, 64:128], in_=ones2_f32[0:1, 0:64])
    nc.vector.tensor_copy(ones2_sb, ones2_f32)
    nc.vector.tensor_copy(ones1_sb[:, 0:64], ones_f32[0:1, 0:64])
    nc.vector.tensor_copy(ones1_sb[:, 64:128], ones_f32[0:1, 0:64])

    # DMA: everything on the sync HWDGE ring (the gpsimd software ring
    # measured only ~50GB/s -- putting wk there gated the whole kernel).
    # The weights are single 3D transfers ([p][dc][col] so each partition
    # line is a contiguous 8KB instead of 8x1KB strided chunks). Order:
    # wk+wq before x (the K/Q chains chase the x chunks), wv mid-x (first
    # V chunk isn't needed until the first attention block), owT last.
    def w3d(w):
        return bass.AP(tensor=w.tensor, offset=w.offset,
                       ap=[[JG, 128], [128 * JG, DC], [1, JG]])

    nc.sync.dma_start(out=wkT_sb[:, 0:DC * JG], in_=w3d(wkT).bitcast(FP32R))
    nc.sync.dma_start(out=wqT_sb[:, 0:DC * JG], in_=w3d(wqT).bitcast(FP32R))
    for jc in range(2):
        nc.sync.dma_start(out=bk_sb[:, jc:jc + 1],
                          in_=bk[jc * 128:(jc + 1) * 128])
        nc.sync.dma_start(out=bq_sb[:, jc:jc + 1],
                          in_=bq[jc * 128:(jc + 1) * 128])
    nc.sync.dma_start(out=bv_row, in_=bv.bitcast(FP32R))
    for dc in range(4):
        nc.sync.dma_start(out=xT_sb[:, dc * S:(dc + 1) * S],
                          in_=xT[dc * 128:(dc + 1) * 128, :].bitcast(FP32R))
    nc.sync.dma_start(out=wvT_sb[:, 0:DC * JG], in_=w3d(wvT).bitcast(FP32R))
    for dc in range(4, DC):
        nc.sync.dma_start(out=xT_sb[:, dc * S:(dc + 1) * S],
                          in_=xT[dc * 128:(dc + 1) * 128, :].bitcast(FP32R))
    for jc in range(2):
        nc.sync.dma_start(out=owT_sb[:, jc * D:(jc + 1) * D],
                          in_=owT[jc * 128:(jc + 1) * 128, :])
    # broadcast bv to all partitions: [1,256] x K=1 ones -> PSUM -> SBUF
    bv_ps = ps.tile((128, JG), FP32, tag="lg01", bufs=1, name="bv_ps")
    nc.tensor.matmul(bv_ps, ones1_sb, bv_row, start=True, stop=True)
    nc.vector.tensor_copy(bv_bc, bv_ps)

    # PSUM tags: av0(1) + av1(1) + lg01(4) + op(1) + vps(1) = 8 banks.
    def proj_chain(w_sb, jc, st, b_sb, dst_sb, tag):
        pp = ps.tile((128, 512), FP32, tag=tag, bufs=1, name=f"pr{tag}")
        for dc in range(DC):
            nc.tensor.matmul(
                pp, w_sb[:, dc * JG + jc * 128: dc * JG + (jc + 1) * 128],
                xT_sb[:, dc * S + st * 512: dc * S + (st + 1) * 512],
                start=(dc == 0), stop=(dc == DC - 1))
        nc.vector.tensor_scalar_add(
            out=dst_sb[:, jc * S + st * 512: jc * S + (st + 1) * 512],
            in0=pp, scalar1=b_sb[:, jc:jc + 1])

    def v_chunk(sc):
        pv = ps.tile((128, JG), FP32, tag="vps", bufs=1, name="vps")
        for dc in range(DC):
            nc.tensor.matmul(
                pv, xT_sb[:, dc * S + sc * 128: dc * S + (sc + 1) * 128],
                wvT_sb[:, dc * JG:(dc + 1) * JG],
                start=(dc == 0), stop=(dc == DC - 1))
        base = sc * 4 * VW
        for a in range(4):
            nc.vector.tensor_add(
                out=Vaug_sb[:, base + a * VW: base + a * VW + HD],
                in0=pv[:, a * HD:(a + 1) * HD],
                in1=bv_bc[:, a * HD:(a + 1) * HD])

    # K jc0 (4 parallel chains) + Q jc0-st0 (5th chain, on the lg01 slot),
    # all dc-outer so the 5 matmuls per dc chase the x DMA chunks ->
    # everything block (qt0, pair0) needs finishes ~one matmul after the
    # last x chunk lands. The K st-chunks complete in query order,
    # matching the kc ranges the block consumes.
    kps = [ps.tile((128, 512), FP32, tag=t, bufs=1, name=f"k{t}")
           for t in ("av0", "av1", "op", "vps")]
    qp = ps.tile((128, 512), FP32, tag="lg01", bufs=1, name="q00")
    for dc in range(DC):
        for st in range(QT_TILES):
            nc.tensor.matmul(
                kps[st], wkT_sb[:, dc * JG: dc * JG + 128],
                xT_sb[:, dc * S + st * 512: dc * S + (st + 1) * 512],
                start=(dc == 0), stop=(dc == DC - 1))
        nc.tensor.matmul(
            qp, wqT_sb[:, dc * JG: dc * JG + 128],
            xT_sb[:, dc * S: dc * S + 512],
            start=(dc == 0), stop=(dc == DC - 1))
    for st in range(QT_TILES):
        nc.vector.tensor_scalar_add(
            out=KT_sb[:, st * 512:(st + 1) * 512],
            in0=kps[st], scalar1=bk_sb[:, 0:1])
    nc.vector.tensor_scalar_add(out=QT_sb[:, 0:512], in0=qp,
                                scalar1=bq_sb[:, 0:1])

    # Side-work drained into the attention blocks' PE slack: the remaining
    # projection chains (op tag) and V chunks (vps tag).
    # pop order: 6 in block 0 (kp2-7), then one per block at kp3 -- each
    # chain must land at least one block before its consumer reads it
    # (block index of Q[jc,st]'s consumer = 2*st + jc).
    side = []
    for st in range(QT_TILES):
        side.append(lambda st=st: proj_chain(wkT_sb, 1, st, bk_sb, KT_sb,
                                             "op"))
    side.append(lambda: proj_chain(wqT_sb, 1, 0, bq_sb, QT_sb, "op"))
    for st in range(1, QT_TILES):
        side.append(lambda st=st: proj_chain(wqT_sb, 0, st, bq_sb, QT_sb,
                                             "op"))
        side.append(lambda st=st: proj_chain(wqT_sb, 1, st, bq_sb, QT_sb,
                                             "op"))
    vleft = list(range(KC))

    def emit_outproj_tile(st, it):
        # alternate between the op and (post-block-0 idle) vps banks so
        # consecutive out-proj tiles don't serialize on one PSUM buffer.
        pp = ps.tile((128, 512), FP32, tag=("op" if it == 0 else "vps"),
                     bufs=1, name="op")
        for jc in range(2):
            nc.tensor.matmul(
                pp,
                attn_outT_sb[:, jc * S + st * 128: jc * S + st * 128 + 128],
                owT_sb[:, jc * D + it * 512: jc * D + (it + 1) * 512],
                start=(jc == 0), stop=(jc == 1))
        ost = p.tile((128, 512), BF16, tag="ost", bufs=4, name="ost")
        nc.vector.tensor_copy(ost, pp)
        nc.sync.dma_start(
            out=out[st * 128:(st + 1) * 128, it * 512:(it + 1) * 512],
            in_=ost)

    def emit_outproj_st(st):
        emit_outproj_tile(st, 0)
        emit_outproj_tile(st, 1)

    def make_normalize(av0, av1, qt, pair):
        # normalize, deferred into the next block so the bcast matmul's
        # dependency chain (DVE copies -> DMA scatter) resolves before it
        # reaches the head of the in-order PE queue. Stage 1 frees the av
        # banks; stage 2 does denominators + muls.
        def stage1():
            raw0 = p.tile((HD, 512), FP32, tag="raw0", bufs=2, name="raw0")
            nc.vector.tensor_copy(raw0, av0[0:HD, :])
            raw1 = p.tile((128, 512), FP32, tag="raw1", bufs=2, name="raw1")
            nc.vector.tensor_copy(raw1[HD:128, :], av1[0:HD, :])
            dstage = p.tile((1, 1024), FP32R, tag="dstage", bufs=2,
                            name="dstage")
            nc.vector.tensor_copy(dstage[:, 0:512], av0[HD:HD + 1, :])
            nc.vector.tensor_copy(dstage[:, 512:1024], av1[HD:HD + 1, :])
            drows = p.tile((2, 512), FP32R, tag="drow", bufs=2, name="drow")
            nc.sync.dma_start(out=drows, in_=dstage)

            def stage2():
                bc = ps.tile((128, 512), FP32, tag="op", bufs=1, name="bc")
                nc.tensor.matmul(bc, ones2_sb, drows, start=True, stop=True)
                rcs = p.tile((128, 512), FP32, tag="rcs", bufs=2, name="rcs")
                nc.vector.reciprocal_approx_fast(rcs, bc)
                base = pair * S + qt * 512
                nc.vector.tensor_mul(
                    out=attn_outT_sb[0:HD, base:base + 512],
                    in0=raw0[0:HD, :], in1=rcs[0:HD, :])
                nc.vector.tensor_mul(
                    out=attn_outT_sb[HD:128, base:base + 512],
                    in0=raw1[HD:128, :], in1=rcs[HD:128, :])
            return stage2
        return stage1

    pending_norm = None
    op_work = []
    for qt in range(QT_TILES):
        for pair in range(2):
            h0 = 2 * pair
            first_block = (qt == 0 and pair == 0)
            av0 = ps.tile((128, 512), FP32, tag="av0", bufs=1, name="av0")
            av1 = ps.tile((128, 512), FP32, tag="av1", bufs=1, name="av1")
            qcol = pair * S + qt * 512

            def emit_avs(at, kc0, kc1):
                for h, avp in ((h0, av0), (h0 + 1, av1)):
                    for i, kc in ((0, kc0), (1, kc1)):
                        nc.tensor.matmul(
                            avp[0:VW, :],
                            Vaug_sb[:, kc * 4 * VW + h * VW:
                                    kc * 4 * VW + (h + 1) * VW],
                            at[:, i * 1024 + (h % 2) * 512:
                               i * 1024 + (h % 2) * 512 + 512],
                            start=(kc == 0), stop=(kc == KC - 1))

            prev = None
            for kp in range(KC // 2):
                kc0, kc1 = 2 * kp, 2 * kp + 1
                if first_block:
                    # V chunks just-in-time: chunk sc is consumed by this
                    # block at kp = sc//2, later blocks reuse Vaug.
                    v_chunk(vleft.pop(0))
                    v_chunk(vleft.pop(0))
                    if kp >= 2 and side:
                        side.pop(0)()
                lg = ps.tile((128, 2048), FP32, tag="lg01", bufs=1,
                             name="lg")
                # bank layout: [h0@kc0 | h1@kc0 | h0@kc1 | h1@kc1] -- each
                # kc's head-pair is adjacent (row-packed concurrent on the
                # PE) AND lives in one exp's bank-half, so the next kp's
                # first pair only WARs the first exp and issues under the
                # second -> the Scalar engine stays back-to-back.
                for i, kc in ((0, kc0), (1, kc1)):
                    kcol = pair * S + kc * 128
                    nc.tensor.matmul(
                        lg[:, i * 1024: i * 1024 + 512],
                        KT_sb[0:64, kcol:kcol + 128],
                        QT_sb[0:64, qcol:qcol + 512],
                        start=True, stop=True, tile_position=(0, 0))
                    nc.tensor.matmul(
                        lg[:, i * 1024 + 512: i * 1024 + 1024],
                        KT_sb[64:128, kcol:kcol + 128],
                        QT_sb[64:128, qcol:qcol + 512],
                        start=True, stop=True, tile_position=(64, 0))
                # The previous kp's AV matmuls are emitted after this kp's
                # exps so they fill the PE during the exp shadow instead of
                # serializing exp -> av -> next-lg.
                at = p.tile((128, 2048), BF16, tag="at", bufs=3, name="at")
                nc.scalar.activation(at[:, 0:1024], lg[:, 0:1024], EXP)
                nc.scalar.activation(at[:, 1024:2048], lg[:, 1024:2048], EXP)
                # stage1 must be emitted before this block's first AV
                # matmuls (kp==1) -- it reads the av banks they overwrite.
                if pending_norm is not None and kp == 0:
                    pending_norm = pending_norm()
                if prev is not None:
                    emit_avs(*prev)
                prev = (at, kc0, kc1)
                # PE filler, emitted after the exps so it runs in the exp
                # shadow: the deferred normalize pieces, one projection
                # side-chain, or one out-proj tile per kc-pair (a batch at
                # a block boundary would bubble the Scalar engine).
                if not first_block:
                    if pending_norm is not None and kp == 2:
                        pending_norm()
                        pending_norm = None
                        if pair == 0 and qt > 0:
                            # qt-1's attn_outT is now fully normalized
                            op_work += [
                                (lambda st=st, it=it:
                                 emit_outproj_tile(st, it))
                                for st in range(4 * (qt - 1), 4 * qt)
                                for it in range(2)]
                    elif kp == 3 and side:
                        side.pop(0)()
                    elif kp >= 3 and op_work:
                        op_work.pop(0)()
            emit_avs(*prev)
            pending_norm = make_normalize(av0, av1, qt, pair)
    # last block's normalize + remaining out-proj tiles
    pending_norm()()
    while op_work:
        op_work.pop(0)()
    for st in range(4 * (QT_TILES - 1), 4 * QT_TILES):
        emit_outproj_st(st)


_NC = None


def _build_nc():
    global _NC
    if _NC is not None:
        return _NC
    nc = bacc.Bacc("TRN2", target_bir_lowering=False, debug=False,
                   num_devices=NCORES)
    xT = nc.dram_tensor("xT", [D, S], FP32, kind="ExternalInput").ap()
    wqT = nc.dram_tensor("wqT", [D, JG], FP32, kind="ExternalInput").ap()
    wkT = nc.dram_tensor("wkT", [D, JG], FP32, kind="ExternalInput").ap()
    wvT = nc.dram_tensor("wvT", [D, JG], FP32, kind="ExternalInput").ap()
    bq = nc.dram_tensor("bq", [JG], FP32, kind="ExternalInput").ap()
    bk = nc.dram_tensor("bk", [JG], FP32, kind="ExternalInput").ap()
    bv = nc.dram_tensor("bv", [JG], FP32, kind="ExternalInput").ap()
    owT = nc.dram_tensor("owT", [JG, D], BF16, kind="ExternalInput").ap()
    out = nc.dram_tensor("out", [S, D], BF16, kind="ExternalOutput").ap()
    with tile.TileContext(nc) as tc:
        mha_core_kernel(tc, out, xT, wqT, wkT, wvT, bq, bk, bv, owT)
    nc.compile()
    _NC = nc
    return nc


def _in_maps(x, kqv_w, kqv_b, out_w):
    maps = []
    for c in range(NCORES):
        b, g = divmod(c, 4)
        sl = slice(g * JG, (g + 1) * JG)
        maps.append({
            "xT": np.ascontiguousarray(x[b].T),
            "wqT": np.ascontiguousarray(kqv_w[0 * D:1 * D][sl].T),
            "wkT": np.ascontiguousarray(kqv_w[1 * D:2 * D][sl].T),
            "wvT": np.ascontiguousarray(kqv_w[2 * D:3 * D][sl].T),
            "bq": np.ascontiguousarray(kqv_b[0 * D:1 * D][sl]),
            "bk": np.ascontiguousarray(kqv_b[1 * D:2 * D][sl]),
            "bv": np.ascontiguousarray(kqv_b[2 * D:3 * D][sl]),
            "owT": np.ascontiguousarray(out_w[:, sl].T).astype(
                ml_dtypes.bfloat16),
        })
    return maps


def run_spmd(x, kqv_w, kqv_b, out_w, out_b, trace=False, tmpdir=None):
    nc = _build_nc()
    res = run_bass_kernel_spmd(nc, _in_maps(x, kqv_w, kqv_b, out_w),
                               list(range(NCORES)), tmpdir=tmpdir, trace=trace)
    parts = [np.asarray(res.results[c]["out"]).astype(np.float32)
             for c in range(NCORES)]
    full = np.stack([
        parts[4 * b] + parts[4 * b + 1] + parts[4 * b + 2] + parts[4 * b + 3]
        + out_b[None, :].astype(np.float32)
        for b in range(B)
    ])
    return full, res


def kernel(**inputs):
    x = np.asarray(inputs["x"], dtype=np.float32)
    kqv_w = np.asarray(inputs["kqv_w"], dtype=np.float32)
    kqv_b = np.asarray(inputs["kqv_b"], dtype=np.float32)
    out_w = np.asarray(inputs["out_w"], dtype=np.float32)
    out_b = np.asarray(inputs["out_b"], dtype=np.float32)
    full, _ = run_spmd(x, kqv_w, kqv_b, out_w, out_b)
    return full
